# revision 57
# baseline (speedup 1.0000x reference)
"""TopK-SAE on 8 TRN2 cores — fp8 DoubleRow encode + host-P sparse decode.

Launch 1 (dict-sharded, ~482us): z0 = fp8e4 DoubleRow encode (W
pre-scaled by 64, fp32 PSUM accumulation) at the fp8 PE roofline (~155
TF/s/core, 216ns per 256x128x512 matmul); per-dict-row top-8 values +
batch indices extracted from PSUM by DVE max8/max_index -> per-core
candidate tables. PE warmup matmuls keep the HAM clock up during the
initial x/W DMA window; the first two d-tiles run batch-major
(interleaved) so compute starts as soon as x chunk 0 + w0 land.
Host: exact merge — all candidates with noisy value >= kth0-DELTA
re-dotted (fp32 einsum + fp64 inside a tight boundary window around the
K-th value), exact global top-K selection; selected latents sorted by
batch row and packed into 16 chunks of 128 batch rows x 3 slot groups
(384 slots/chunk; overflow latents applied directly on host).
Launch 2 (A-sharded, ~42us): x_hatT slice += G_g.T @ P_g per slot
group, with the one-hot*act P matrices prebuilt on host and DMAed
partition-major (no on-device mask build). fp16 output, upcast + b_dec
on host. All device DMAs use multi-KB per-partition lines (DMA cost on
TRN2 is per-descriptor = per partition line).
"""
import numpy as np

B, A, D, K = 2048, 4096, 32768, 4096
NCORES = 8
DL = D // NCORES            # dict rows per core
DT = DL // 128              # d-tiles per core
KT2 = A // 256              # DoubleRow contraction tiles
BCH = 512                   # encode matmul free-dim chunk
NBCH = B // BCH
CHUNKS = [(0, 512), (512, 512), (1024, 512), (1536, 512)]
DCH = 128                   # decode batch chunk (rows)
NDCH = B // DCH             # 16 decode chunks
NGC = 3                     # slot groups per decode chunk
CAPC = NGC * 128            # 384 slots per chunk
NSLOT = NDCH * CAPC         # 6144 total slots
NGRP = NSLOT // 128         # 48 slot groups
AS = A // NCORES            # A-shard per core
AT = AS // 128
WSCALE = 64.0
DELTA = 0.25                # fp8 z0 noise band (sigma ~0.0375, max ~0.21)
FP64_WND = 0.002            # fp64 re-dot window around the boundary

_CACHE = {}


def build_enc():
    import concourse.bacc as bacc
    import concourse.mybir as mybir
    from concourse import tile

    f32 = mybir.dt.float32
    f8 = mybir.dt.float8e4
    u32 = mybir.dt.uint32
    Act = mybir.ActivationFunctionType
    DR = mybir.MatmulPerfMode.DoubleRow

    nc = bacc.Bacc("TRN2", target_bir_lowering=False, debug=False,
                   num_devices=NCORES)
    xps = [nc.dram_tensor(f"xp{c}", [128, KT2 * 2 * w], f8,
                          kind="ExternalInput")
           for c, (_, w) in enumerate(CHUNKS)]
    wp = nc.dram_tensor("wp", [DT, 128, KT2 * 2 * 128], f8,
                        kind="ExternalInput")
    benc = nc.dram_tensor("benc", [DL, 1], f32, kind="ExternalInput")
    cand_v = nc.dram_tensor("cand_v", [128, DT * 8], f32,
                            kind="ExternalOutput")
    cand_i = nc.dram_tensor("cand_i", [128, DT * 8], u32,
                            kind="ExternalOutput")

    benc_r = benc.rearrange("(d p) c -> p (d c)", p=128)

    with tile.TileContext(nc) as tc:
        with (
            tc.tile_pool(name="uni", bufs=1) as unip,
            tc.tile_pool(name="wt", bufs=3) as wtp,
            tc.tile_pool(name="sm", bufs=2) as smp,
            tc.tile_pool(name="ps", bufs=2, space="PSUM") as pse,
        ):
            benc_sb = unip.tile([128, DT], f32, tag="benc", name="benc")
            nc.sync.dma_start(benc_sb[:], benc_r)
            cv = unip.tile([128, DT * 8], f32, tag="cv", name="cv")
            ci = unip.tile([128, DT * 8], u32, tag="ci", name="ci")
            # PE warmup while input DMAs stream (keeps HAM at full clock)
            wupa = unip.tile([128, 128], mybir.dt.float16, tag="wua",
                             name="wua")
            nc.any.memset(wupa[:], 0.0)
            wups = pse.tile([128, B], f32, tag="zps", name="wups")
            for _ in range(200):
                nc.tensor.matmul(wups[:, 0:128], wupa[:], wupa[:],
                                 start=True, stop=True)
            # DMA priority: x chunk 0 + w0 first; later transfers follow.
            xts = [unip.tile([128, KT2 * 2 * w], f8, tag=f"x{c}",
                             name=f"x{c}")
                   for c, (_, w) in enumerate(CHUNKS)]
            WF = KT2 * 2 * 128
            XF0 = KT2 * 2 * CHUNKS[0][1]
            for q in range(6):
                sl = slice(q * XF0 // 6, (q + 1) * XF0 // 6)
                nc.sync.dma_start(xts[0][:, sl], xps[0][:, sl])
            wpre = []
            for d in range(2):
                wth = wtp.tile([128, KT2 * 2 * 128], f8, tag="wt", name="wt")
                if d == 0:
                    for q in range(2):
                        sl = slice(q * WF // 2, (q + 1) * WF // 2)
                        nc.sync.dma_start(wth[:, sl], wp[d, :, sl])
                else:
                    nc.sync.dma_start(wth[:], wp[d, :, :])
                wpre.append(wth)
            # x1 whole; x2/x3 as kt-halves so bridge chains can start on
            # half-arrived chunks (finer DMA dependency granularity)
            nc.sync.dma_start(xts[1][:], xps[1][:, :])
            XH = KT2 * CHUNKS[2][1]          # half the flat columns
            for c in (2, 3):
                for h in range(2):
                    nc.sync.dma_start(xts[c][:, h * XH:(h + 1) * XH],
                                      xps[c][:, h * XH:(h + 1) * XH])
            xvs = [t[:].rearrange("p (kt ko c) -> p kt ko c", ko=2,
                                  c=CHUNKS[i][1])
                   for i, t in enumerate(xts)]
            wvpre = [w[:].rearrange("p (kt ko m) -> p kt ko m", ko=2, m=128)
                     for w in wpre]

            def dve(d, zps):
                mv = smp.tile([128, 8], f32, tag="mv", name="mv")
                nc.vector.max(mv[:], zps[:])
                nc.vector.max_index(ci[:, d * 8:(d + 1) * 8], mv[:], zps[:])
                nc.scalar.activation(cv[:, d * 8:(d + 1) * 8], mv[:],
                                     Act.Relu, bias=benc_sb[:, d:d + 1],
                                     scale=1.0 / WSCALE)

            # bridge: d0/d1 interleaved batch-major chains track x arrival.
            # d1 finishes second-to-last and owns the PSUM buffer d2 will
            # take, so d1's DVE drains it during d0's final chain.
            zt = [pse.tile([128, B], f32, tag="zps", name=f"zps{d}")
                  for d in range(2)]
            zfor = {0: zt[1], 1: zt[0]}
            nch = len(CHUNKS)
            for c, (off, w) in enumerate(CHUNKS):
                dorder = (0, 1) if c < nch - 1 else (1, 0)
                for d in dorder:
                    for kt in range(KT2):
                        nc.tensor.matmul(
                            zfor[d][:, off:off + w],
                            wvpre[d][:, kt], xvs[c][:, kt],
                            start=(kt == 0), stop=(kt == KT2 - 1),
                            perf_mode=DR)
            dve(1, zfor[1])
            dve(0, zfor[0])
            for d in range(2, DT):
                wth = wtp.tile([128, KT2 * 2 * 128], f8, tag="wt",
                               name="wt")
                nc.sync.dma_start(wth[:], wp[d, :, :])
                wv = wth[:].rearrange("p (kt ko m) -> p kt ko m",
                                      ko=2, m=128)
                zps = pse.tile([128, B], f32, tag="zps", name="zps")
                if d == 2:
                    order = [(kt, c) for c in range(len(CHUNKS))
                             for kt in range(KT2)]
                else:
                    order = [(kt, c) for kt in range(KT2)
                             for c in range(len(CHUNKS))]
                for kt, c in order:
                    off, w = CHUNKS[c]
                    nc.tensor.matmul(
                        zps[:, off:off + w],
                        wv[:, kt], xvs[c][:, kt],
                        start=(kt == 0), stop=(kt == KT2 - 1),
                        perf_mode=DR)
                dve(d, zps)
                if d == DT - 2:
                    nc.sync.dma_start(cand_v[:, :(DT - 1) * 8],
                                      cv[:, :(DT - 1) * 8])
                    nc.sync.dma_start(cand_i[:, :(DT - 1) * 8],
                                      ci[:, :(DT - 1) * 8])
            nc.sync.dma_start(cand_v[:, (DT - 1) * 8:],
                              cv[:, (DT - 1) * 8:])
            nc.sync.dma_start(cand_i[:, (DT - 1) * 8:],
                              ci[:, (DT - 1) * 8:])
    nc.compile()
    return nc


def build_dec(ranges):
    """ranges: per decode chunk, (g0, g1) slot-group span (tight pack).

    Slots are the K selected latents sorted by batch row, no padding.
    A boundary group is processed by both adjacent chunks; its P block
    differs per chunk (instance), so P is stored per (chunk, group)
    instance while G is stored once per group (4.2MB vs 6.3MB padded).
    """
    import concourse.bacc as bacc
    import concourse.mybir as mybir
    from concourse import tile

    f32 = mybir.dt.float32
    f16 = mybir.dt.float16

    NGT = K // 128               # 32 tight slot groups
    GQ = 8                       # G split into 8 piece DMAs
    GPQ = NGT // GQ              # groups per piece
    NGI = sum(g1 - g0 for g0, g1 in ranges)   # P instances
    # P piece boundaries: instances of 4 chunks per piece
    pbound = [0]
    acc = 0
    for n, (g0, g1) in enumerate(ranges):
        acc += g1 - g0
        if n % 4 == 3:
            pbound.append(acc)

    nc = bacc.Bacc("TRN2", target_bir_lowering=False, debug=False,
                   num_devices=NCORES)
    Gin = nc.dram_tensor("Gin", [GQ, 128, GPQ * AS], f16,
                         kind="ExternalInput")
    Pin = nc.dram_tensor("Pin", [128, NGI * DCH], f16,
                         kind="ExternalInput")
    out = nc.dram_tensor("out", [128, AT * B], f16, kind="ExternalOutput")
    out_r = out.rearrange("p (at b) -> p at b", b=B)

    with tile.TileContext(nc) as tc:
        with (
            tc.tile_pool(name="uni", bufs=1) as unip,
            tc.tile_pool(name="ps", bufs=2, space="PSUM") as psd,
        ):
            # PE warmup while G/P DMAs stream
            wupa = unip.tile([128, 128], f16, tag="wua", name="wua")
            nc.any.memset(wupa[:], 0.0)
            wups = psd.tile([128, DCH], f32, tag="dps0", name="wups")
            for _ in range(24):
                nc.tensor.matmul(wups[:], wupa[:], wupa[:],
                                 start=True, stop=True)
            gts = [unip.tile([128, GPQ * AS], f16, tag=f"g{q}",
                             name=f"g{q}") for q in range(GQ)]
            pt = unip.tile([128, NGI * DCH], f16, tag="pt", name="pt")
            # consumption-ordered issue: G pieces with P pieces interleaved;
            # the first G/P pieces are split so chunk 0's deps land first
            g0sp = 3 * AS                        # first 3 groups of piece 0
            nc.sync.dma_start(gts[0][:, :g0sp], Gin[0, :, :g0sp])
            p0sp = (ranges[0][1] - ranges[0][0]
                    + ranges[1][1] - ranges[1][0]) * DCH  # chunks 0-1 insts
            nc.sync.dma_start(pt[:, :p0sp], Pin[:, :p0sp])
            nc.sync.dma_start(gts[0][:, g0sp:], Gin[0, :, g0sp:])
            nc.sync.dma_start(pt[:, p0sp:pbound[1] * DCH],
                              Pin[:, p0sp:pbound[1] * DCH])
            for q in range(1, GQ):
                if q % 2 == 0 and q // 2 < len(pbound) - 1:
                    k = q // 2
                    nc.sync.dma_start(
                        pt[:, pbound[k] * DCH:pbound[k + 1] * DCH],
                        Pin[:, pbound[k] * DCH:pbound[k + 1] * DCH])
                nc.sync.dma_start(gts[q][:], Gin[q, :, :])
            osb = unip.tile([128, AT * B], f16, tag="osb", name="osb")
            osb_r = osb[:].rearrange("p (at b) -> p at b", b=B)
            inst = 0
            for n in range(NDCH):
                g0, g1 = ranges[n]
                dpss = [psd.tile([128, DCH], f32, tag=f"dps{at}",
                                 name=f"dps{at}")
                        for at in range(AT)]
                for j, g in enumerate(range(g0, g1)):
                    q, lg = g // GPQ, g % GPQ
                    for at in range(AT):
                        nc.tensor.matmul(
                            dpss[at][:],
                            gts[q][:, lg * AS + at * 128:
                                   lg * AS + at * 128 + 128],
                            pt[:, inst * DCH:(inst + 1) * DCH],
                            start=(j == 0), stop=(j == g1 - g0 - 1))
                    inst += 1
                for at in range(AT):
                    nc.vector.tensor_copy(
                        osb[:, at * B + n * DCH:at * B + (n + 1) * DCH],
                        dpss[at][:])
                if n % 4 == 3 and n != NDCH - 1:
                    q0 = (n - 3) * DCH
                    nc.sync.dma_start(out_r[:, :, q0:q0 + 4 * DCH],
                                      osb_r[:, :, q0:q0 + 4 * DCH])
                elif n == NDCH - 3:
                    q0 = (NDCH - 4) * DCH
                    nc.sync.dma_start(out_r[:, :, q0:q0 + 2 * DCH],
                                      osb_r[:, :, q0:q0 + 2 * DCH])
            q0 = (NDCH - 2) * DCH
            nc.sync.dma_start(out_r[:, :, q0:q0 + 2 * DCH],
                              osb_r[:, :, q0:q0 + 2 * DCH])
    nc.compile()
    return nc


def _get_enc():
    if "enc" not in _CACHE:
        _CACHE["enc"] = build_enc()
    return _CACHE["enc"]


def _get_dec(ranges):
    key = ("dec", ranges)
    if key not in _CACHE:
        _CACHE[key] = build_dec(ranges)
    return _CACHE[key]


def _pack_x2(xa):
    import ml_dtypes
    x8t = np.ascontiguousarray(xa.T).astype(ml_dtypes.float8_e4m3)  # [A, B]
    arr = x8t.reshape(KT2, 2, 128, B).transpose(2, 0, 1, 3)  # [128,kt,ko,B]
    return [np.ascontiguousarray(
        arr[:, :, :, off:off + w]).reshape(128, KT2 * 2 * w)
        for off, w in CHUNKS]


def _pack_w(Wc):
    """Wc: [DL, A] fp32 core shard -> [DT, 128, KT2*2*128] fp8 (x64)."""
    import ml_dtypes
    w8t = np.ascontiguousarray(Wc.T * WSCALE).astype(ml_dtypes.float8_e4m3)
    arr = w8t.reshape(KT2, 2, 128, DT, 128).transpose(3, 2, 0, 1, 4)
    return np.ascontiguousarray(arr).reshape(DT, 128, KT2 * 2 * 128)


def kernel(x, W_enc, b_enc, W_dec, b_dec):
    from concourse.bass_utils import run_bass_kernel_spmd

    x = np.asarray(x, np.float32)
    W_enc = np.asarray(W_enc, np.float32)
    b_enc = np.asarray(b_enc, np.float32)
    W_dec = np.asarray(W_dec, np.float32)
    b_dec = np.asarray(b_dec, np.float32)
    nc_enc = _get_enc()

    xa = x - b_dec[None, :]
    xps = _pack_x2(xa)
    in1 = []
    for i in range(NCORES):
        sl = slice(i * DL, (i + 1) * DL)
        m = {f"xp{c}": xps[c] for c in range(len(CHUNKS))}
        m["wp"] = _pack_w(W_enc[sl])
        m["benc"] = np.ascontiguousarray(b_enc[sl]).reshape(DL, 1)
        in1.append(m)
    r1 = run_bass_kernel_spmd(nc_enc, in1, core_ids=list(range(NCORES)))

    # ---- host merge: per-dict-row top-8 candidates -> exact global top-K --
    dloc = (np.arange(128)[:, None]
            + 128 * (np.arange(DT * 8)[None, :] // 8))
    cv = np.stack([r1.results[c]["cand_v"] for c in range(NCORES)])
    bi = np.stack([r1.results[c]["cand_i"].astype(np.int64)
                   for c in range(NCORES)])
    dg = (dloc[None, :, :] + (np.arange(NCORES) * DL)[:, None, None])
    cvf, bif, dgf = cv.ravel(), bi.ravel(), dg.ravel()
    kth0 = np.partition(cvf, -K)[-K]
    uni = np.nonzero(cvf >= kth0 - DELTA)[0]
    ub, ud = bif[uni], dgf[uni]
    # max_index can emit duplicate (b, d) pairs on exact value ties
    _, ufirst = np.unique(ub * np.int64(D) + ud, return_index=True)
    ub, ud = ub[ufirst], ud[ufirst]
    v32 = (np.einsum("ij,ij->i", W_enc[ud], xa[ub], optimize=True)
           + b_enc[ud])
    kth32 = np.partition(v32, -K)[-K]
    wnd = np.abs(v32 - kth32) <= FP64_WND
    if wnd.any():
        wi = np.nonzero(wnd)[0]
        v32 = v32.astype(np.float64)
        v32[wi] = (np.einsum("ij,ij->i", W_enc[ud[wi]].astype(np.float64),
                             xa[ub[wi]].astype(np.float64))
                   + b_enc[ud[wi]])
    order = np.argsort(-v32)[:K]
    acts = np.maximum(v32[order], 0.0).astype(np.float32)
    rows_b = ub[order]
    cols_d = ud[order]

    # ---- sort by batch row; tight-pack slots, data-dependent ranges ----
    srt = np.argsort(rows_b, kind="stable")
    acts, rows_b, cols_d = acts[srt], rows_b[srt], cols_d[srt]
    cbound = np.searchsorted(rows_b, np.arange(NDCH + 1) * DCH)
    NGT = K // 128
    ranges = []
    for n in range(NDCH):
        g0 = int(cbound[n]) // 128
        g1 = max(-(-int(cbound[n + 1]) // 128), g0 + 1)
        g1 = min(g1, NGT)
        g0 = min(g0, g1 - 1)
        ranges.append((g0, g1))
    ranges = tuple(ranges)
    nc_dec = _get_dec(ranges)

    Wsel = W_dec[cols_d].astype(np.float16)                # [K, A]
    # P instances: one [128, DCH] block per (chunk, group) pair
    blocks = []
    for n, (g0, g1) in enumerate(ranges):
        for g in range(g0, g1):
            r = rows_b[g * 128:(g + 1) * 128].astype(np.int64) - n * DCH
            a = acts[g * 128:(g + 1) * 128]
            blk = np.zeros((128, DCH), np.float16)
            m = (r >= 0) & (r < DCH)
            blk[np.nonzero(m)[0], r[m]] = a[m].astype(np.float16)
            blocks.append(blk)
    NGI = len(blocks)
    Pin = np.ascontiguousarray(
        np.stack(blocks).transpose(1, 0, 2)).reshape(128, NGI * DCH)
    GQ = 8
    GPQ = NGT // GQ
    Wg = Wsel.reshape(GQ, GPQ, 128, A).transpose(0, 2, 1, 3)
    in2 = [{"Gin": np.ascontiguousarray(
                Wg[:, :, :, c * AS:(c + 1) * AS]).reshape(GQ, 128, GPQ * AS),
            "Pin": Pin} for c in range(NCORES)]
    r2 = run_bass_kernel_spmd(nc_dec, in2, core_ids=list(range(NCORES)))

    xhatT = np.empty((A, B), np.float32)
    for c in range(NCORES):
        o = r2.results[c]["out"]                           # [128, AT*B]
        xhatT[c * AS:(c + 1) * AS, :] = (
            o.reshape(128, AT, B).transpose(1, 0, 2).reshape(AS, B))
    return np.ascontiguousarray(xhatT.T) + b_dec[None, :]
